# revision 7
# baseline (speedup 1.0000x reference)
"""NF4-quantized LoRA linear layer on 8 Trainium2 NeuronCores.

Computation (reference):
    w = NF4_TABLE[w_codes] * w_scales[block-expanded]        # [O, I]
    out = x @ w.T + (alpha/rank) * (x @ lora_a.T) @ lora_b.T # [B, S, O]

Strategy:
  - Tensor-parallel split of the output dim across 8 cores (O_SH = 512 each).
    Every core sees all of x; no collectives; host concatenates outputs.
  - LoRA folded into the weights per i-tile on the PE (la.T @ lb), with those
    matmuls doubling as the HAM warm-up burst at t=0.
  - NF4 dequant: reduced approximate chain (max table err ~1e-3, well inside
    the 2e-2 gate): one linear tensor_scalar + 6 ACT relu ramps + 1 step,
    combined with 7 DVE adds, then *scales and +lora.  ~3x cheaper than an
    exact chain, so the PE is fed from ~11us instead of ~90us.
  - The m-loop runs in 3 i-phases (3/11/18 i-tiles).  Phase partials live in
    SBUF as bf16 (no DRAM round-trip).  Dequant of later macros is emitted
    interleaved with the m-loop so engine FIFOs never head-block.
"""

import numpy as np
import ml_dtypes

import concourse.mybir as mybir
import concourse.tile as tile
from concourse import bacc
from concourse.bass_utils import run_bass_kernel_spmd

B, S, I, O, R, BLK = 4, 2048, 4096, 4096, 16, 64
M = B * S                      # 8192 token rows
N_CORES = 8
O_SH = O // N_CORES            # 512 output cols per core
IT = I // 128                  # 32 contraction tiles
MT = M // 128                  # 64 row tiles
LORA_SCALE = 2.0               # alpha / rank

# i-tiles per dequant macro; prefix sums give macro it-ranges
MACROS = [3, 3, 4, 4, 4, 4, 4, 4, 2]
PHASES = [(0, 3), (3, 14), (14, 32)]   # i-tile ranges per m-loop phase
N_WARM = 8                              # dummy warm-up matmuls

# Reduced NF4 chain: t(c) ~= a + b*c + sum_j g_j*relu(c - v_j) + d*[c>=4.5]
# constants least-squares fit + f16-greedy-polished (max err 9.8e-4)
CH_A = -0.9989130932850444
CH_B = 0.30380719897117314
CH_D = 0.0049800797507012625
CH_U = 4.5
CH_RAMPS = [
    (-0.03645944185100243, 3.458792198605635),
    (0.011337839111885224, 9.536136365753613),
    (-0.013258293109855572, 6.776118162286423),
    (-0.17365163570191863, 1.2338965634872725),
    (0.030104234814644314, 11.63488034466834),
    (0.1558432629796038, 13.748487442766368),
]

F16 = mybir.dt.float16
BF16 = mybir.dt.bfloat16
F32 = mybir.dt.float32
ALU = mybir.AluOpType
ACTF = mybir.ActivationFunctionType

BF16_NP = ml_dtypes.bfloat16


def _macro_ranges():
    out, lo = [], 0
    for n in MACROS:
        out.append((lo, lo + n))
        lo += n
    return out


def _build_nc():
    nc = bacc.Bacc("TRN2", target_bir_lowering=False, debug=False,
                   num_devices=N_CORES)

    xt = nc.dram_tensor("xt", [128, MT, IT, 128], BF16, kind="ExternalInput")
    codes = nc.dram_tensor("codes", [I, O_SH], F16, kind="ExternalInput")
    scales = nc.dram_tensor("scales", [I, O_SH], F16, kind="ExternalInput")
    la = nc.dram_tensor("la", [R, I], BF16, kind="ExternalInput")
    lb = nc.dram_tensor("lb", [R, O_SH], BF16, kind="ExternalInput")
    out = nc.dram_tensor("out", [M, O_SH], F32, kind="ExternalOutput")

    codes_r = codes.ap().rearrange("(t p) o -> p t o", p=128)
    scales_r = scales.ap().rearrange("(t p) o -> p t o", p=128)
    mranges = _macro_ranges()

    with tile.TileContext(nc) as tc:
        with (
            tc.tile_pool(name="wpool", bufs=len(MACROS)) as wpool,
            tc.tile_pool(name="wlpool", bufs=4) as wlpool,
            tc.tile_pool(name="dqio", bufs=3) as dqio,
            tc.tile_pool(name="dq", bufs=2) as dq,
            tc.tile_pool(name="xpool", bufs=3) as xpool,
            tc.tile_pool(name="cpool", bufs=1) as cpool,
            tc.tile_pool(name="opool", bufs=2) as opool,
            tc.tile_pool(name="ps_a", bufs=2, space="PSUM") as pp_a,
            tc.tile_pool(name="ps_b", bufs=2, space="PSUM") as pp_b,
            tc.tile_pool(name="ps_c", bufs=2, space="PSUM") as pp_c,
            tc.tile_pool(name="ps_l", bufs=2, space="PSUM") as pp_l,
        ):
            pp_phase = [pp_a, pp_b, pp_c]

            # ---- constants ----
            la_sb = cpool.tile([R, I], BF16, tag="la")
            nc.gpsimd.dma_start(la_sb[:], la.ap())
            lb_sb = cpool.tile([R, O_SH], BF16, tag="lb")
            nc.gpsimd.dma_start(lb_sb[:], lb.ap())
            # per-ramp bias constants (-|g|*v) for ACT relu
            biases = cpool.tile([128, len(CH_RAMPS)], F32, tag="bias")
            for j, (g, v) in enumerate(CH_RAMPS):
                nc.vector.memset(biases[:, j:j + 1], -abs(g) * v)
            # SBUF bf16 partial accumulator [128, MT*512]
            pa = cpool.tile([128, MT * O_SH], BF16, tag="pa")

            # ---- wl (lora weight fold) + dummy warm-up on the PE ----
            wl_tiles = {}

            def emit_wl(mi):
                it_lo, it_hi = mranges[mi]
                nt = it_hi - it_lo
                wl = wlpool.tile([128, nt * O_SH], F16, tag="wl")
                for j in range(nt):
                    it = it_lo + j
                    pl = pp_l.tile([128, O_SH], F32, tag="pl")
                    nc.tensor.matmul(
                        pl[:], la_sb[:, it * 128:(it + 1) * 128], lb_sb[:],
                        start=True, stop=True,
                    )
                    nc.scalar.copy(wl[:, j * O_SH:(j + 1) * O_SH], pl[:])
                wl_tiles[mi] = wl

            for mi in range(4):
                emit_wl(mi)
            for _ in range(N_WARM):
                pl = pp_l.tile([128, O_SH], F32, tag="pl")
                nc.tensor.matmul(
                    pl[:], la_sb[:, 0:128], la_sb[:, 0:O_SH],
                    start=True, stop=True,
                )

            # ---- dequant: DMA + chain emission helpers ----
            w_aps = {}

            def emit_macro_dma(mi):
                it_lo, it_hi = mranges[mi]
                nt = it_hi - it_lo
                fd = nt * O_SH
                ct = dqio.tile([128, fd], F16, tag="ct")
                nc.gpsimd.dma_start(
                    ct[:].rearrange("p (t o) -> p t o", t=nt),
                    codes_r[:, it_lo:it_hi, :],
                )
                st = dqio.tile([128, fd], F16, tag="st")
                nc.gpsimd.dma_start(
                    st[:].rearrange("p (t o) -> p t o", t=nt),
                    scales_r[:, it_lo:it_hi, :],
                )
                return ct, st

            slots = {}   # mi -> (ct, st), filled by the dma closure

            def chain_ops(mi):
                """Closures, one engine-op each, for macro mi's chain."""
                it_lo, it_hi = mranges[mi]
                nt = it_hi - it_lo
                fd = nt * O_SH
                state = {}
                ops = []

                def op_lin():
                    ct, _ = slots[mi]
                    acc = dq.tile([128, fd], F16, tag="acc")
                    nc.vector.tensor_scalar(
                        acc[:], ct[:], CH_B, CH_A, op0=ALU.mult, op1=ALU.add)
                    state["acc"] = acc
                ops.append(op_lin)
                for j, (g, v) in enumerate(CH_RAMPS):
                    def op_ramp(j=j, g=g):
                        ct, _ = slots[mi]
                        r = dq.tile([128, fd], F16, tag="rmp")
                        nc.scalar.activation(
                            r[:], ct[:], ACTF.Relu,
                            bias=biases[:, j:j + 1], scale=abs(g))
                        state["r"] = r
                    ops.append(op_ramp)

                    def op_comb(g=g):
                        acc = state["acc"]
                        nc.vector.tensor_tensor(
                            acc[:], acc[:], state["r"][:],
                            op=ALU.add if g > 0 else ALU.subtract)
                    ops.append(op_comb)

                def op_step():
                    ct, _ = slots[mi]
                    stp = dq.tile([128, fd], F16, tag="stp")
                    nc.vector.tensor_scalar(
                        stp[:], ct[:], CH_U, CH_D, op0=ALU.is_ge, op1=ALU.mult)
                    state["stp"] = stp
                ops.append(op_step)

                def op_addstep():
                    acc = state["acc"]
                    nc.vector.tensor_tensor(
                        acc[:], acc[:], state["stp"][:], op=ALU.add)
                ops.append(op_addstep)

                def op_scale():
                    _, st = slots[mi]
                    acc = state["acc"]
                    nc.vector.tensor_tensor(acc[:], acc[:], st[:], op=ALU.mult)
                ops.append(op_scale)

                def op_lora():
                    wt = wpool.tile([128, fd], BF16, tag="w")
                    nc.vector.tensor_tensor(
                        wt[:], state["acc"][:], wl_tiles[mi][:], op=ALU.add)
                    for j, it in enumerate(range(it_lo, it_hi)):
                        w_aps[it] = wt[:, j * O_SH:(j + 1) * O_SH]
                ops.append(op_lora)
                return ops

            def dma_op(mi):
                def do_dma():
                    slots[mi] = emit_macro_dma(mi)
                return [do_dma]

            pending = []

            def pump(n):
                for _ in range(n):
                    if pending:
                        pending.pop(0)()

            # macros 0-1 fully upfront (phase A weights + head start on B)
            pending += dma_op(0) + dma_op(1) + chain_ops(0) + chain_ops(1)
            pump(len(pending))
            # macro 2-3 chains pumped through the phase-A loop, each chain
            # preceded by the NEXT macro's dma so the DVE never head-blocks
            # on an in-flight codes/scales transfer
            pending += dma_op(2) + dma_op(3) + chain_ops(2)
            pending += dma_op(4) + chain_ops(3)

            next_macro = [4]

            def emit_phase(ph):
                i_lo, i_hi = PHASES[ph]
                n_it = i_hi - i_lo
                for mt in range(MT):
                    xa = xpool.tile([128, n_it, 128], BF16, tag=f"x{ph}")
                    nc.sync.dma_start(xa[:], xt.ap()[:, mt, i_lo:i_hi, :])
                    po = pp_phase[ph].tile([128, O_SH], F32, tag=f"p{ph}")
                    for k, it in enumerate(range(i_lo, i_hi)):
                        nc.tensor.matmul(
                            po[:], xa[:, k, :], w_aps[it],
                            start=(k == 0), stop=(k == n_it - 1),
                        )
                    pslice = pa[:, mt * O_SH:(mt + 1) * O_SH]
                    if ph == 0:
                        nc.scalar.copy(pslice, po[:])
                    elif ph == 1:
                        nc.vector.tensor_tensor(
                            pslice, po[:], pslice, op=ALU.add)
                    else:
                        ev = opool.tile([128, O_SH], F32, tag="ev")
                        nc.vector.tensor_tensor(
                            ev[:], po[:], pslice, op=ALU.add)
                        nc.sync.dma_start(
                            out.ap()[mt * 128:(mt + 1) * 128, :], ev[:])
                    # pace the dequant stream through the loop
                    if ph == 0 and mt % 2 == 0:
                        pump(1)
                    elif ph == 1:
                        if mt in (0, 13, 26, 39, 52) and next_macro[0] < len(MACROS):
                            mi = next_macro[0]
                            if mi + 1 < len(MACROS):
                                pending.extend(dma_op(mi + 1))
                            pending.extend(chain_ops(mi))
                            next_macro[0] += 1
                        pump(1)
                pump(len(pending))

            emit_phase(0)
            for mi in range(4, len(MACROS)):
                emit_wl(mi)
            emit_phase(1)
            emit_phase(2)

    nc.compile()
    return nc


_NC_CACHE = {}


def _get_nc():
    if "nc" not in _NC_CACHE:
        _NC_CACHE["nc"] = _build_nc()
    return _NC_CACHE["nc"]


def prepare_in_maps(x, w_codes, w_scales, lora_a, lora_b):
    """Host-side sharding + layout prep (no arithmetic beyond casts/folds)."""
    xm = np.ascontiguousarray(x.reshape(M, I))
    # xt[p, mt, t, mm] = x[mt*128+mm, t*128+p], bf16
    xtl = (
        xm.T.reshape(IT, 128, MT, 128)
        .transpose(1, 2, 0, 3)
        .astype(BF16_NP)
    )
    xtl = np.ascontiguousarray(xtl)

    la = np.ascontiguousarray(
        (LORA_SCALE * lora_a.astype(np.float64)).astype(BF16_NP)
    )

    in_maps = []
    for c in range(N_CORES):
        o_lo, o_hi = c * O_SH, (c + 1) * O_SH
        codes_t = np.ascontiguousarray(
            w_codes[o_lo:o_hi].T.astype(np.float16)
        )
        scales_t = np.ascontiguousarray(
            np.repeat(w_scales[o_lo:o_hi].T, BLK, axis=0).astype(np.float16)
        )
        lb_t = np.ascontiguousarray(lora_b[o_lo:o_hi].T.astype(BF16_NP))
        in_maps.append(
            {
                "xt": xtl,
                "codes": codes_t,
                "scales": scales_t,
                "la": la,
                "lb": lb_t,
            }
        )
    return in_maps


def run(in_maps, trace=False, retries=2):
    nc = _get_nc()
    last = None
    for attempt in range(retries + 1):
        try:
            return run_bass_kernel_spmd(
                nc, in_maps, core_ids=list(range(N_CORES)), trace=trace
            )
        except Exception as e:  # transient NRT/axon device errors
            last = e
            if attempt == retries:
                raise
            import time as _time

            _time.sleep(5)
    raise last


def kernel(x, w_codes, w_scales, lora_a, lora_b):
    in_maps = prepare_in_maps(x, w_codes, w_scales, lora_a, lora_b)
    res = run(in_maps, trace=False)
    out = np.concatenate(
        [res.results[c]["out"] for c in range(N_CORES)], axis=1
    )
    return out.reshape(B, S, O).astype(np.float32)
